# revision 30
# baseline (speedup 1.0000x reference)
"""DegradationAttention TRN2 kernel.

Math (faithful to the reference):
    q, k are the *memory-reinterpreting* reshape of [B,L,H,E] -> [B*H, L, E]
    (mixes L and H exactly like torch .view on a contiguous tensor), v is the
    true per-head slice values[b, :, h, :].
    d2      = |q_l|^2 + |k_s|^2 - 2 q_l.k_s           (>= 0 mathematically)
    scores  = 1 - exp(-d2); causal mask; A = softmax(scores / 8)
    out     = A @ v

Implementation notes:
  * d2 is produced by ONE matmul via host-side augmentation:
        khat = [k, |k|^2, 1]  (66 x S),  qhat = [-2q, 1, |q|^2]  (66 x L)
    so khat^T @ qhat = d2^T (s on partitions, l free).  No max-subtraction is
    needed in the softmax: unmasked scaled scores are bounded in [0, 0.125]
    for ANY input, so exp never overflows.  The softmax numerator is
        exp(0.125*(1 - e1)) with e1 = exp(-d2)
    (the reference's max-subtraction only shifts by a row constant which
    cancels in the normalization).  Masked entries are zeroed by a 0/1
    upper-triangular multiply on the 16 diagonal 128x128 blocks only; all
    fully-masked blocks are never computed (causal tiling halves the work).
  * A ones-column appended to V makes the AV matmul also emit the softmax
    row-sums, so normalization is a [128,1] reciprocal + scale on the output.
  * batch*heads = 16 slices -> 2 per NeuronCore, no cross-core communication.
"""

from contextlib import ExitStack

import ml_dtypes
import numpy as np

import concourse.mybir as mybir
import concourse.tile as tile
from concourse import bacc
from concourse.bass_utils import run_bass_kernel_spmd
from concourse.masks import make_upper_triangular

B, L, S, H, E, D = 2, 2048, 2048, 8, 64, 64
N_CORES = 8
HPC = (B * H) // N_CORES  # head-slices per core = 2
NJ = S // 128  # s-chunks per head = 16
KAUG = E + 2  # 66
VW = D + 1  # 65: V plus ones column
SCALE = 0.125  # 1/sqrt(E)

# Column offset of A^T chunk j inside the per-head A buffer.  Chunk j holds
# columns l in [128*j, L) (the causally-reachable l for s-chunk j).
_A_OFFS = []
_o = 0
for _j in range(NJ):
    _A_OFFS.append(_o)
    _o += L - 128 * _j
A_COLS = _o  # 17408

TRACE = False  # test.py sets True to collect an NTFF profile
LAST = {}  # exec_time_ns etc. from the most recent run

# Pass-2 of the softmax numerator, exp(0.125*(1-e1)) for e1 in [0,1], is a
# near-linear function.  Minimax-linear on [0,1]: rel err <= 1.04e-3 before
# normalization; the row-common component cancels in the softmax division
# (exactly 0 error when e1 is constant along a row), and the residual is far
# below the bf16 rounding of A that the matmul dtype already imposes.  One
# 4x-mode DVE tensor_scalar per tile instead of a second ACT exp pass.
_LIN_B = -0.13314845306682632
_LIN_A = 1.1321084564415727

SEG = 1024  # score-segment width (2 PSUM banks)

_CACHE = {}


def _build_program():
    nc = bacc.Bacc(
        "TRN2", target_bir_lowering=False, debug=False, num_devices=N_CORES
    )
    bf16 = mybir.dt.bfloat16
    f32 = mybir.dt.float32
    AF = mybir.ActivationFunctionType

    qh_d = nc.dram_tensor("qhat", [HPC, KAUG, L], bf16, kind="ExternalInput").ap()
    kh_d = nc.dram_tensor("khat", [HPC, KAUG, S], bf16, kind="ExternalInput").ap()
    vh_d = nc.dram_tensor("vhat", [HPC, 2, 128, NJ * VW], bf16, kind="ExternalInput").ap()
    out_d = nc.dram_tensor("out", [HPC, L, D], f32, kind="ExternalOutput").ap()

    with tile.TileContext(nc) as tc, ExitStack() as ctx:
        consts = ctx.enter_context(tc.tile_pool(name="consts", bufs=1))
        io = ctx.enter_context(tc.tile_pool(name="io", bufs=2))
        apool = ctx.enter_context(tc.tile_pool(name="apool", bufs=2))
        epool = ctx.enter_context(tc.tile_pool(name="epool", bufs=6))
        opool = ctx.enter_context(tc.tile_pool(name="opool", bufs=3))
        ps_s = ctx.enter_context(tc.tile_pool(name="ps_s", bufs=3, space="PSUM"))
        ps_o = ctx.enter_context(tc.tile_pool(name="ps_o", bufs=2, space="PSUM"))

        mask = consts.tile([128, 128], bf16, tag="mask")
        # mask[s, l] = 1 where l >= s (keep), else 0
        make_upper_triangular(nc, mask[:], val=1.0, diag=True)

        class Head:
            """Per-head tiles + emission helpers."""

            def __init__(self, h):
                self.h = h
                self.kh = io.tile([KAUG, S], bf16, tag="kh")
                self.qh = io.tile([KAUG, L], bf16, tag="qh")
                self.vh = io.tile([128, NJ * VW], bf16, tag="vh")
                self.vl = io.tile([128, NJ * VW], bf16, tag="vl")
                # split loads along the boundaries chunk-0 consumes so the
                # first score segment starts as early as possible
                nc.gpsimd.dma_start(out=self.kh[:, 0:128], in_=kh_d[h, :, 0:128])
                nc.sync.dma_start(out=self.qh[:, 0:SEG], in_=qh_d[h, :, 0:SEG])
                nc.gpsimd.dma_start(out=self.kh[:, 128:], in_=kh_d[h, :, 128:])
                nc.sync.dma_start(out=self.qh[:, SEG:], in_=qh_d[h, :, SEG:])
                nc.sync.dma_start(out=self.vh[:, 0:VW], in_=vh_d[h, 0, :, 0:VW])
                nc.sync.dma_start(out=self.vh[:, VW:], in_=vh_d[h, 0, :, VW:])
                nc.sync.dma_start(out=self.vl[:], in_=vh_d[h, 1])
                self.A = apool.tile([128, A_COLS], bf16, tag="A")
                self.stage = opool.tile([128, NJ, D], f32, tag="stage")

            def scores(self, t, l0, w, ps, pcol):
                """score matmuls for chunk t, l in [l0, l0+w), into
                ps[:, pcol:pcol+w] (one matmul per 512-wide PSUM bank run)."""
                s0 = 128 * t
                for b0 in range(0, w, 512):
                    bw = min(512, w - b0)
                    nc.tensor.matmul(
                        ps[:, pcol + b0 : pcol + b0 + bw],
                        self.kh[:, s0 : s0 + 128],
                        self.qh[:, l0 + b0 : l0 + b0 + bw],
                        start=True,
                        stop=True,
                    )

            def lin_to_A(self, e1, ecol, t, l0, w):
                acol = _A_OFFS[t] + (l0 - 128 * t)
                nc.vector.tensor_scalar(
                    self.A[:, acol : acol + w],
                    e1[:, ecol : ecol + w],
                    _LIN_B,
                    _LIN_A,
                    mybir.AluOpType.mult,
                    mybir.AluOpType.add,
                )

            def mask_diag(self, t):
                nc.vector.tensor_mul(
                    self.A[:, _A_OFFS[t] : _A_OFFS[t] + 128],
                    self.A[:, _A_OFFS[t] : _A_OFFS[t] + 128],
                    mask[:],
                )

            def produce(self, t):
                """scores chunk t -> exp -> linear map -> mask."""
                l0 = 128 * t
                while l0 < L:
                    w = min(SEG, L - l0)
                    ps = ps_s.tile([128, SEG], f32, tag="ps_s")
                    self.scores(t, l0, w, ps, 0)
                    e1 = epool.tile([128, SEG], bf16, tag="e1")
                    nc.scalar.activation(e1[:, :w], ps[:, :w], AF.Exp, scale=-1.0)
                    self.lin_to_A(e1, 0, t, l0, w)
                    l0 += w
                self.mask_diag(t)

            def produce_pair(self, ta, wa, tb, wb):
                """pack two narrow chunks (wa+wb <= SEG, wa bank-multiple)
                into one PSUM tile -> single exp instruction."""
                ps = ps_s.tile([128, SEG], f32, tag="ps_s")
                self.scores(ta, 128 * ta, wa, ps, 0)
                self.scores(tb, 128 * tb, wb, ps, wa)
                e1 = epool.tile([128, SEG], bf16, tag="e1")
                nc.scalar.activation(
                    e1[:, : wa + wb], ps[:, : wa + wb], AF.Exp, scale=-1.0
                )
                if _A_OFFS[ta] + wa == _A_OFFS[tb]:  # adjacent in A
                    self.lin_to_A(e1, 0, ta, 128 * ta, wa + wb)
                else:
                    self.lin_to_A(e1, 0, ta, 128 * ta, wa)
                    self.lin_to_A(e1, wa, tb, 128 * tb, wb)
                self.mask_diag(ta)
                self.mask_diag(tb)

            def av_block(self, t):
                """AV for l-block t (accumulate hi+lo products over s-chunks
                j <= t), normalize by the ones-column row-sum, stage + DMA."""
                po = ps_o.tile([128, VW], f32, tag="po")
                for j in range(t + 1):
                    acol = _A_OFFS[j] + 128 * (t - j)
                    nc.tensor.matmul(
                        po[:],
                        self.A[:, acol : acol + 128],
                        self.vh[:, VW * j : VW * (j + 1)],
                        start=(j == 0),
                        stop=False,
                    )
                    nc.tensor.matmul(
                        po[:],
                        self.A[:, acol : acol + 128],
                        self.vl[:, VW * j : VW * (j + 1)],
                        start=False,
                        stop=(j == t),
                    )
                r = opool.tile([128, 1], f32, tag="r")
                nc.vector.reciprocal(r[:], po[:, D : D + 1])
                nc.vector.tensor_scalar_mul(self.stage[:, t, :], po[:, 0:D], r[:])
                if t % 4 == 3:  # stream results out every 4 l-blocks
                    nc.sync.dma_start(
                        out=out_d[self.h].rearrange("(t p) d -> p t d", p=128)[
                            :, t - 3 : t + 1, :
                        ],
                        in_=self.stage[:, t - 3 : t + 1, :],
                    )

        # Heads run sequentially, but head 1's first score chunks are hoisted
        # into head 0's AV-heavy tail so ACT has exp work across the head
        # boundary (its DMAs start mid-head-0).
        h0 = Head(0)
        h1 = None
        for t in range(12):
            h0.produce(t)
            h0.av_block(t)
            if t == 5:
                h1 = Head(1)
        h1.produce(0)
        h0.produce_pair(12, 512, 13, 384)  # contiguous in A
        h0.av_block(12)
        h0.av_block(13)
        h1.produce(1)
        h1.produce(2)
        h0.produce_pair(14, 256, 15, 128)
        h0.av_block(14)
        h0.av_block(15)
        h1.av_block(0)
        h1.av_block(1)
        h1.av_block(2)
        for t in range(3, 12):
            h1.produce(t)
            h1.av_block(t)
        h1.produce_pair(12, 512, 13, 384)
        h1.av_block(12)
        h1.av_block(13)
        h1.produce_pair(14, 256, 15, 128)
        h1.av_block(14)
        h1.av_block(15)

    nc.compile()
    return nc


def _prep_inputs(queries, keys, values):
    """Host-side augmentation; returns per-core input maps."""
    q = np.ascontiguousarray(np.asarray(queries, dtype=np.float32)).reshape(
        B * H, L, E
    )
    k = np.ascontiguousarray(np.asarray(keys, dtype=np.float32)).reshape(B * H, S, E)
    v = np.asarray(values, dtype=np.float32).transpose(0, 2, 1, 3).reshape(B * H, S, D)

    qq = np.einsum("nle,nle->nl", q, q)
    kk = np.einsum("nse,nse->ns", k, k)

    qhat = np.empty((B * H, KAUG, L), dtype=np.float32)
    qhat[:, :E, :] = -2.0 * q.transpose(0, 2, 1)
    qhat[:, E, :] = 1.0
    qhat[:, E + 1, :] = qq

    khat = np.empty((B * H, KAUG, S), dtype=np.float32)
    khat[:, :E, :] = k.transpose(0, 2, 1)
    khat[:, E, :] = kk
    khat[:, E + 1, :] = 1.0

    vfull = np.empty((B * H, S, VW), dtype=np.float32)
    vfull[:, :, :D] = v
    vfull[:, :, D] = 1.0
    # [n, S, VW] -> [n, 128, NJ*VW] with element (p, j*VW+d) = vfull[n, j*128+p, d]
    vfull = np.ascontiguousarray(
        vfull.reshape(B * H, NJ, 128, VW).transpose(0, 2, 1, 3).reshape(
            B * H, 128, NJ * VW
        )
    )

    bf = ml_dtypes.bfloat16
    qhat = qhat.astype(bf)
    khat = khat.astype(bf)
    # split-precision V: V = hi + lo with both halves bf16 restores ~fp32
    # accuracy in the AV matmul (the A-weights' bf16 rounding cancels in the
    # softmax normalization)
    v_hi = vfull.astype(bf)
    v_lo = (vfull - v_hi.astype(np.float32)).astype(bf)
    vhat = np.ascontiguousarray(np.stack([v_hi, v_lo], axis=1))

    in_maps = []
    for c in range(N_CORES):
        sl = slice(HPC * c, HPC * (c + 1))
        in_maps.append(
            {
                "qhat": np.ascontiguousarray(qhat[sl]),
                "khat": np.ascontiguousarray(khat[sl]),
                "vhat": np.ascontiguousarray(vhat[sl]),
            }
        )
    return in_maps


def kernel(queries, keys, values):
    if "nc" not in _CACHE:
        _CACHE["nc"] = _build_program()
    nc = _CACHE["nc"]

    in_maps = _prep_inputs(queries, keys, values)
    try:
        res = run_bass_kernel_spmd(
            nc,
            in_maps,
            core_ids=list(range(N_CORES)),
            trace=TRACE,
        )
    except ModuleNotFoundError:
        # NTFF profiling hook unavailable in this environment
        res = run_bass_kernel_spmd(
            nc, in_maps, core_ids=list(range(N_CORES)), trace=False
        )
    LAST["exec_time_ns"] = res.exec_time_ns
    LAST["mean_exec_time_ns"] = res.mean_exec_time_ns

    out = np.concatenate([r["out"] for r in res.results], axis=0)  # [B*H, L, D]
    out = out.reshape(B, H, L, D).transpose(0, 2, 1, 3)  # [B, L, H, D]
    return np.ascontiguousarray(out)
